# revision 8
# baseline (speedup 1.0000x reference)
"""Trainium2 Bass kernel: autoregressive GRU decoder (nn_Decoder).

B=1024, T=128, H=1024, I=128 (POSE=96 + TRAJ=32).
Data-parallel over batch across 8 NeuronCores (128 rows/core), no collectives.

Layout: fully transposed on-device — features on partitions, batch on the
free dim. h state kept as 8 K-tiles [128, 128]; x state [128, 128].
Matmul operands bf16, state fp32, PSUM accumulation fp32.

Feature order on the I axis is permuted to [pose(0:96), traj(96:128)] inside
the kernel (original is [traj, pose]) so the pose matmul (M=96) can target
partition base 0 and traj (M=32) base 96. Host applies/undoes the permutation.
"""

import sys

if "/opt/trn_rl_repo" not in sys.path:
    sys.path.insert(0, "/opt/trn_rl_repo")

import numpy as np
import ml_dtypes

B, T, H = 1024, 128, 1024
POSE, TRAJ = 96, 32
I = POSE + TRAJ  # 128
NCORES = 8
BL = B // NCORES  # 128 batch rows per core
KH = H // 128  # 8 h K-tiles
P = 128

# kernel feature k -> original feature perm[k]
_PERM = np.concatenate([np.arange(TRAJ, I), np.arange(0, TRAJ)])  # [pose, traj]
_PERM_INV = np.argsort(_PERM)

# chunks (in units of 128-wide k-tiles) for the elementwise gate pipeline
_CHUNKS = [(0, 4), (4, 7), (7, 8)]

_BUILD_CACHE = {}
LAST_RESULTS = None


def _build(t_steps):
    import concourse.bass as bass
    import concourse.tile as tile
    from concourse import bacc, mybir

    f32 = mybir.dt.float32
    bf16 = mybir.dt.bfloat16
    AF = mybir.ActivationFunctionType
    OP = mybir.AluOpType

    nc = bacc.Bacc(None, target_bir_lowering=False, debug=False)

    # ---- DRAM I/O ------------------------------------------------------
    dp = nc.declare_dram_parameter
    x0_d = dp("x0", [P, BL], f32, isOutput=False)             # x0^T (permuted)
    h0_d = dp("h0", [P, KH, BL], f32, isOutput=False)         # h0^T k-tiles
    wrz_d = dp("wrz", [P, 9, 16, P], bf16, isOutput=False)    # [p,k,m,j] k0=x
    wnx_d = dp("wnx", [P, KH, P], bf16, isOutput=False)       # Win^T
    wnh_d = dp("wnh", [P, KH, KH, P], bf16, isOutput=False)   # Whn^T [p,k,m,j]
    wlp_d = dp("wlp", [P, KH, POSE], bf16, isOutput=False)    # lp_W^T
    wfcp_d = dp("wfcp", [POSE, TRAJ], bf16, isOutput=False)   # fc pose part^T
    wfch_d = dp("wfch", [P, KH, TRAJ], bf16, isOutput=False)  # fc h part^T
    brz_d = dp("brz", [P, 16], f32, isOutput=False)           # col m = bias m-tile
    bxn_d = dp("bxn", [P, KH], f32, isOutput=False)
    bhn_d = dp("bhn", [P, KH], f32, isOutput=False)
    blp_d = dp("blp", [POSE, 1], f32, isOutput=False)
    btp_d = dp("btp", [P, 1], f32, isOutput=False)            # [lp_b; fc_b]
    yt_d = dp("yt", [t_steps, P, BL], f32, isOutput=True)     # y^T per step

    with tile.TileContext(nc) as tc:
        with (
            tc.tile_pool(name="const", bufs=1) as cpool,
            tc.tile_pool(name="state", bufs=2) as spool,
            tc.tile_pool(name="work", bufs=2) as wpool,
            tc.tile_pool(name="gates_ps", bufs=3, space="PSUM") as gpool,
            tc.tile_pool(name="tp_ps", bufs=2, space="PSUM") as tpool,
        ):
            # ---- one-time loads ----------------------------------------
            def load_const(dram, shape, dtype):
                t = cpool.tile(shape, dtype, tag=dram.name)
                nc.sync.dma_start(t[:], dram[:])
                return t

            wrz_s = load_const(wrz_d, [P, 9, 16, P], bf16)
            wnx_s = load_const(wnx_d, [P, KH, P], bf16)
            wnh_s = load_const(wnh_d, [P, KH, KH, P], bf16)
            wlp_s = load_const(wlp_d, [P, KH, POSE], bf16)
            wfcp_s = load_const(wfcp_d, [POSE, TRAJ], bf16)
            wfch_s = load_const(wfch_d, [P, KH, TRAJ], bf16)
            brz_s = load_const(brz_d, [P, 16], f32)
            bxn_s = load_const(bxn_d, [P, KH], f32)
            bhn_s = load_const(bhn_d, [P, KH], f32)
            blp_s = load_const(blp_d, [POSE, 1], f32)
            btp_s = load_const(btp_d, [P, 1], f32)

            h_f = spool.tile([P, KH, BL], f32, tag="hf")
            nc.sync.dma_start(h_f[:], h0_d[:])
            h_b = spool.tile([P, KH, BL], bf16, tag="hb")
            nc.vector.tensor_copy(h_b[:], h_f[:])
            x_f = spool.tile([P, BL], f32, tag="xf")
            nc.sync.dma_start(x_f[:], x0_d[:])
            x_b = spool.tile([P, BL], bf16, tag="xb")
            nc.vector.tensor_copy(x_b[:], x_f[:])

            # ---- time steps --------------------------------------------
            for t in range(t_steps):
                # r/z pre-activations: 16 m-tiles, K = x(1) + h(8) tiles.
                # x K-tile last so the previous step's y latency is hidden.
                ps_r = gpool.tile([P, KH, BL], f32, tag="ps")
                ps_z = gpool.tile([P, KH, BL], f32, tag="ps")
                for mi, ps in ((0, ps_r), (8, ps_z)):
                    for m in range(KH):
                        out = ps[:, m, :]
                        for k in range(KH):
                            nc.tensor.matmul(
                                out, wrz_s[:, 1 + k, mi + m, :], h_b[:, k, :],
                                start=(k == 0), stop=False,
                            )
                        nc.tensor.matmul(
                            out, wrz_s[:, 0, mi + m, :], x_b[:],
                            start=False, stop=True,
                        )

                ps_xn = gpool.tile([P, KH, BL], f32, tag="ps")
                for m in range(KH):
                    nc.tensor.matmul(
                        ps_xn[:, m, :], wnx_s[:, m, :], x_b[:],
                        start=True, stop=True,
                    )
                ps_hn = gpool.tile([P, KH, BL], f32, tag="ps")
                for m in range(KH):
                    out = ps_hn[:, m, :]
                    for k in range(KH):
                        nc.tensor.matmul(
                            out, wnh_s[:, k, m, :], h_b[:, k, :],
                            start=(k == 0), stop=(k == KH - 1),
                        )

                # gates
                r_s = wpool.tile([P, KH, BL], f32, tag="r")
                z_s = wpool.tile([P, KH, BL], f32, tag="z")
                for m in range(KH):
                    nc.scalar.activation(
                        r_s[:, m, :], ps_r[:, m, :], AF.Sigmoid,
                        bias=brz_s[:, m : m + 1],
                    )
                for m in range(KH):
                    nc.scalar.activation(
                        z_s[:, m, :], ps_z[:, m, :], AF.Sigmoid,
                        bias=brz_s[:, KH + m : KH + m + 1],
                    )
                # zc = 1 - z ; f = z * h   (gpsimd, off the critical tail)
                zc = wpool.tile([P, KH, BL], f32, tag="zc")
                nc.gpsimd.tensor_scalar(
                    zc[:], z_s[:], -1.0, 1.0, OP.mult, OP.add
                )
                f_s = wpool.tile([P, KH, BL], f32, tag="f")
                nc.gpsimd.tensor_mul(f_s[:], z_s[:], h_f[:])

                # t1 = (hn + bhn) * r ; t2 = (xn + bxn) + t1
                t1 = wpool.tile([P, KH, BL], f32, tag="t1")
                t2 = wpool.tile([P, KH, BL], f32, tag="t2")
                for m in range(KH):
                    nc.vector.scalar_tensor_tensor(
                        t1[:, m, :], ps_hn[:, m, :], bhn_s[:, m : m + 1],
                        r_s[:, m, :], op0=OP.add, op1=OP.mult,
                    )
                for m in range(KH):
                    nc.vector.scalar_tensor_tensor(
                        t2[:, m, :], ps_xn[:, m, :], bxn_s[:, m : m + 1],
                        t1[:, m, :], op0=OP.add, op1=OP.add,
                    )

                # n = tanh(t2); h' = n*zc + f; cast to bf16 — chunked so the
                # tail after the last hn matmul stays short.
                n_s = wpool.tile([P, KH, BL], f32, tag="n")
                h_f2 = spool.tile([P, KH, BL], f32, tag="hf")
                h_b2 = spool.tile([P, KH, BL], bf16, tag="hb")
                u_s = wpool.tile([P, KH, BL], f32, tag="u")
                for c0, c1 in _CHUNKS:
                    nc.scalar.activation(
                        n_s[:, c0:c1, :], t2[:, c0:c1, :], AF.Tanh
                    )
                    nc.vector.tensor_mul(
                        u_s[:, c0:c1, :], n_s[:, c0:c1, :], zc[:, c0:c1, :]
                    )
                    nc.vector.tensor_add(
                        h_f2[:, c0:c1, :], u_s[:, c0:c1, :], f_s[:, c0:c1, :]
                    )
                    nc.vector.tensor_copy(h_b2[:, c0:c1, :], h_f2[:, c0:c1, :])

                # pose / traj / y
                ps_pose = tpool.tile([P, BL], f32, tag="tp")
                for k in range(KH):
                    nc.tensor.matmul(
                        ps_pose[0:POSE, :], wlp_s[:, k, :], h_b2[:, k, :],
                        start=(k == 0), stop=(k == KH - 1),
                    )
                pose_b = wpool.tile([POSE, BL], bf16, tag="pose")
                nc.scalar.activation(
                    pose_b[:], ps_pose[0:POSE, :], AF.Identity,
                    bias=blp_s[:, 0:1],
                )
                ps_traj = tpool.tile([P, BL], f32, tag="tp")
                for k in range(KH):
                    nc.tensor.matmul(
                        ps_traj[POSE:P, :], wfch_s[:, k, :], h_b2[:, k, :],
                        start=(k == 0), stop=False, tile_position=(0, 96),
                    )
                nc.tensor.matmul(
                    ps_traj[POSE:P, :], wfcp_s[:, :], pose_b[:],
                    start=False, stop=True, tile_position=(0, 96),
                )

                # y = x + tp + btp  (pose rows then traj rows), y becomes x
                x_f2 = spool.tile([P, BL], f32, tag="xf")
                nc.vector.scalar_tensor_tensor(
                    x_f2[0:POSE, :], ps_pose[0:POSE, :], btp_s[0:POSE, 0:1],
                    x_f[0:POSE, :], op0=OP.add, op1=OP.add,
                )
                nc.vector.scalar_tensor_tensor(
                    x_f2[POSE:P, :], ps_traj[POSE:P, :], btp_s[POSE:P, 0:1],
                    x_f[POSE:P, :], op0=OP.add, op1=OP.add,
                )
                x_b2 = spool.tile([P, BL], bf16, tag="xb")
                nc.vector.tensor_copy(x_b2[:], x_f2[:])
                nc.sync.dma_start(yt_d[t, :, :], x_f2[:])

                x_f, x_b, h_f, h_b = x_f2, x_b2, h_f2, h_b2

    nc.compile()
    return nc


def _prep_inputs(h, gt, Wih, Whh, bih, bhh, lp_W, lp_b, fc_W, fc_b):
    """Host-side: permute features, transpose, cast weights to bf16."""
    bf = ml_dtypes.bfloat16
    f32 = np.float32

    Wih_k = np.ascontiguousarray(Wih[:, _PERM])  # [3H, I] cols permuted

    # rz combined weights, transposed: [1152, 2048] -> [p, k(9), m(16), j]
    wrzT = np.concatenate(
        [Wih_k[: 2 * H].T, Whh[: 2 * H].T], axis=0
    )  # [I+H, 2H]; rows 0:128 = x part
    wrz = np.empty((P, 9, 16, P), dtype=bf)
    for k in range(9):
        for m in range(16):
            wrz[:, k, m, :] = wrzT[k * P : (k + 1) * P, m * P : (m + 1) * P]

    wnxT = Wih_k[2 * H :].T  # [128, 1024]
    wnx = np.ascontiguousarray(wnxT.reshape(P, KH, P), dtype=bf)  # [p, m, j]

    wnhT = Whh[2 * H :].T  # [1024, 1024]
    wnh = np.empty((P, KH, KH, P), dtype=bf)
    for k in range(KH):
        for m in range(KH):
            wnh[:, k, m, :] = wnhT[k * P : (k + 1) * P, m * P : (m + 1) * P]

    wlpT = lp_W.T  # [1024, 96]
    wlp = np.ascontiguousarray(
        wlpT.reshape(KH, P, POSE).transpose(1, 0, 2), dtype=bf
    )  # [p, k, 96]

    wfcpT = np.ascontiguousarray(fc_W[:, :POSE].T, dtype=bf)  # [96, 32]
    wfchT = fc_W[:, POSE:].T  # [1024, 32]
    wfch = np.ascontiguousarray(
        wfchT.reshape(KH, P, TRAJ).transpose(1, 0, 2), dtype=bf
    )  # [p, k, 32]

    b_rz = (bih + bhh)[: 2 * H].astype(f32)  # [2048]
    brz = np.ascontiguousarray(b_rz.reshape(16, P).T)  # [128, 16]
    bxn = np.ascontiguousarray(bih[2 * H :].reshape(KH, P).T.astype(f32))
    bhn = np.ascontiguousarray(bhh[2 * H :].reshape(KH, P).T.astype(f32))
    blp = lp_b.reshape(POSE, 1).astype(f32)
    btp = np.concatenate([lp_b, fc_b]).reshape(P, 1).astype(f32)

    shared = {
        "wrz": wrz, "wnx": wnx, "wnh": wnh, "wlp": wlp,
        "wfcp": wfcpT, "wfch": wfch, "brz": brz, "bxn": bxn,
        "bhn": bhn, "blp": blp, "btp": btp,
    }

    in_maps = []
    for c in range(NCORES):
        sl = slice(c * BL, (c + 1) * BL)
        x0 = np.ascontiguousarray(gt[sl, 0, :][:, _PERM].T.astype(f32))  # [I,BL]
        h0 = np.ascontiguousarray(
            h[sl, :].T.reshape(KH, P, BL).transpose(1, 0, 2).astype(f32)
        )  # [p, k, b] = h[b, k*128+p]
        in_maps.append({"x0": x0, "h0": h0, **shared})
    return in_maps


def kernel(h, gt, Wih, Whh, bih, bhh, lp_W, lp_b, fc_W, fc_b, time_steps):
    from concourse.bass_utils import run_bass_kernel_spmd

    t_steps = int(time_steps)

    h = np.asarray(h, np.float32)
    gt = np.asarray(gt, np.float32)

    if t_steps not in _BUILD_CACHE:
        _BUILD_CACHE[t_steps] = _build(t_steps)
    nc = _BUILD_CACHE[t_steps]

    in_maps = _prep_inputs(
        h, gt, np.asarray(Wih, np.float32), np.asarray(Whh, np.float32),
        np.asarray(bih, np.float32), np.asarray(bhh, np.float32),
        np.asarray(lp_W, np.float32), np.asarray(lp_b, np.float32),
        np.asarray(fc_W, np.float32), np.asarray(fc_b, np.float32),
    )

    import os

    trace = bool(os.environ.get("KERNEL_TRACE"))
    res = run_bass_kernel_spmd(
        nc, in_maps, core_ids=list(range(NCORES)), trace=trace
    )
    global LAST_RESULTS
    LAST_RESULTS = res

    out = np.empty((B, t_steps, I), dtype=np.float32)
    for c in range(NCORES):
        yt = res.results[c]["yt"]  # [T, I_k, BL]
        out[c * BL : (c + 1) * BL] = yt.transpose(2, 0, 1)[:, :, _PERM_INV]
    return out
